# revision 49
# baseline (speedup 1.0000x reference)
"""DeepseekV2 MLA decoder-layer attention on 8 Trainium2 NeuronCores.

Distribution (tensor-parallel over heads, per the source hint):
  - A-projection (hidden @ w_qkv_a.T) is sequence-sharded: each core computes
    the fused low-rank latents for its 256-token shard, applies the rmsnorms
    (ln weights folded into the B-projection weights on host) and the k_pe
    RoPE, then one AllGather replicates the full latent matrix.
  - B-projections, RoPE(q_pe), flash-style causal attention and o_proj are
    head-sharded: core c owns heads {2c, 2c+1}; its o_proj against the matching
    w_o column slice yields a partial [2048, 2048] output.
  - Unshard on host: output = sum of the 8 partials (RowParallel reduction).

All matmuls run in float32r (~1.7e-4 scale-relative max error per 2048-deep
contraction, full PE throughput).

Layout conventions on device (partition dim first):
  activations feature-major [d, s] so matmuls contract on partitions;
  v is token-major [t, (h, vdim)] so PV contracts over keys;
  scores are computed transposed [t_chunk, s_block]; softmax row sums via
  ones-vector matmuls; normalization deferred to after PV.
"""
import numpy as np

import concourse.bass as bass
import concourse.mybir as mybir
import concourse.tile as tile
from concourse import bacc
from concourse.bass_utils import run_bass_kernel_spmd

HIDDEN = 2048
H = 16
NOPE = 128
ROPE = 64
VDIM = 128
QLR = 1536
KVLR = 512
QK = NOPE + ROPE            # 192
THETA = 10000.0
EPS = 1e-6
SEQ = 2048

N_CORES = 8
HPC = H // N_CORES          # 2 heads per core
SSH = SEQ // N_CORES        # 256-token shard
LAT_C = 17                  # latent chunks: 12 q_a + 4 kv_a + 1 (kpe, 64 rows)
P = 128

F32 = mybir.dt.float32
F32R = mybir.dt.float32r
BF16 = mybir.dt.bfloat16
FLASH_DT = F32R             # dtype of q/k/v/exp inside flash attention

SCALE = float(QK) ** -0.5
NEG = -1.0e30

N_KC = HIDDEN // P          # 16
N_QAC = QLR // P            # 12
N_KVC = KVLR // P           # 4
N_SB = SEQ // 512           # 4 query blocks
N_SC = SEQ // P             # 16


def build_program():
    nc = bacc.Bacc("TRN2", target_bir_lowering=False, debug=False,
                   num_devices=N_CORES)

    h1 = nc.dram_tensor("h1", [P, N_KC, SSH], F32R, kind="ExternalInput")
    w1 = nc.dram_tensor("w1", [LAT_C, P, HIDDEN], F32R, kind="ExternalInput")
    wq = nc.dram_tensor("wq", [P, N_QAC, HPC * QK], F32R, kind="ExternalInput")
    wkv = nc.dram_tensor("wkv", [P, N_KVC, HPC * (NOPE + VDIM)], F32R, kind="ExternalInput")
    wo = nc.dram_tensor("wo", [P, HPC, HIDDEN], F32R, kind="ExternalInput")
    cosq = nc.dram_tensor("cosq", [ROPE, SEQ], F32R, kind="ExternalInput")
    ssinq = nc.dram_tensor("ssinq", [ROPE, SEQ], F32R, kind="ExternalInput")
    cosl = nc.dram_tensor("cosl", [ROPE, SSH], F32R, kind="ExternalInput")
    ssinl = nc.dram_tensor("ssinl", [ROPE, SSH], F32R, kind="ExternalInput")
    pswap = nc.dram_tensor("pswap", [ROPE, ROPE], F32R, kind="ExternalInput")
    onesc_d = nc.dram_tensor("onesc", [P, 1], F32R, kind="ExternalInput")
    onesr_d = nc.dram_tensor("onesr", [1, P], F32R, kind="ExternalInput")
    yout = nc.dram_tensor("y", [SEQ, HIDDEN], F32, kind="ExternalOutput")

    with tile.TileContext(nc) as tc:
        _emit(nc, tc, h1, w1, wq, wkv, wo, cosq, ssinq, cosl, ssinl, pswap,
              onesc_d, onesr_d, yout)
    nc.compile()
    return nc


def _emit(nc, tc, h1, w1, wq, wkv, wo, cosq, ssinq, cosl, ssinl, pswap,
          onesc_d, onesr_d, yout):
    Exp = mybir.ActivationFunctionType.Exp
    Sqrt = mybir.ActivationFunctionType.Sqrt
    rg = [list(range(N_CORES))]

    with tc.tile_pool(name="const", bufs=1) as const, \
         tc.tile_pool(name="work", bufs=2) as work, \
         tc.tile_pool(name="lstr", bufs=3) as lstr, \
         tc.tile_pool(name="epool", bufs=3) as epool, \
         tc.tile_pool(name="psum", bufs=1, space="PSUM") as psum, \
         tc.tile_pool(name="dram", bufs=1, space="DRAM") as dram:

        # PSUM tags (8 banks): qacc x3, oacc, oacc2, zacc, zacc2, bcast
        def ps(shape, tag, name, bufs=None):
            return psum.tile(shape, F32, tag=tag, name=name, bufs=bufs)

        # ---- constants ----
        ones_col_t = const.tile([P, 1], F32R)
        nc.sync.dma_start(ones_col_t[:], onesc_d[:])
        ones_col = ones_col_t[:]
        ones_row_t = const.tile([1, P], F32R)
        nc.sync.dma_start(ones_row_t[:], onesr_d[:])
        ones_row = ones_row_t[:]
        psw = const.tile([ROPE, ROPE], F32R)
        nc.sync.dma_start(psw[:], pswap[:])
        eps1 = const.tile([1, 1], F32)
        nc.vector.memset(eps1[:], EPS)
        masks = const.tile([P, 4, 512], F32)
        for j in range(4):
            nc.vector.memset(masks[:, j, :], 0.0)
            nc.gpsimd.affine_select(
                out=masks[:, j, :], in_=masks[:, j, :],
                compare_op=mybir.AluOpType.is_ge, fill=NEG,
                base=-128 * j, pattern=[[1, 512]], channel_multiplier=-1,
            )

        # ================= phase 1: local A-proj + rmsnorm + kpe rope =======
        with tc.tile_pool(name="ph1", bufs=1) as ph1:
            hloc = ph1.tile([P, N_KC, SSH], F32R)
            nc.sync.dma_start(hloc[:], h1[:])
            lat = ph1.tile([P, LAT_C, SSH], F32R)
            ss_q = ps([1, SSH], "zacc", "ss_q")
            ss_kv = ps([1, SSH], "zacc2", "ss_kv")

            for m in range(17):
                acc = ps([P, SSH], "qacc", "a_acc", bufs=3)
                wt = ph1.tile([P, HIDDEN], F32R, name="w1t", bufs=3)
                nc.sync.dma_start(wt[:], w1[m])
                for k in range(N_KC):
                    nc.tensor.matmul(acc[:], wt[:, k * P:(k + 1) * P], hloc[:, k, :],
                                     start=(k == 0), stop=(k == N_KC - 1))
                nc.vector.tensor_copy(lat[:, m, :], acc[:])
                if m < 16:
                    sq = work.tile([P, SSH], F32R, name="sq")
                    nc.scalar.square(sq[:], acc[:])
                    tgt = ss_q if m < N_QAC else ss_kv
                    nc.tensor.matmul(tgt[:], ones_col[:], sq[:],
                                     start=(m == 0) or (m == N_QAC),
                                     stop=(m == N_QAC - 1) or (m == 15))

            for tag, ss, nchunk, mbase, denom in (
                ("q", ss_q, N_QAC, 0, QLR), ("kv", ss_kv, N_KVC, N_QAC, KVLR),
            ):
                rt = work.tile([1, SSH], F32, name=f"rt{tag}", bufs=1)
                nc.scalar.activation(rt[:], ss[:], Sqrt, bias=eps1[:], scale=1.0 / denom)
                ri = work.tile([1, SSH], F32R, name=f"ri{tag}", bufs=1)
                with nc.allow_low_precision(reason="float32r is bitwise float32"):
                    nc.vector.reciprocal(ri[:], rt[:])
                bc = ps([P, SSH], "bcast", f"bc{tag}")
                nc.tensor.matmul(bc[:], ones_row[:], ri[:], start=True, stop=True)
                bcs = work.tile([P, SSH], F32, name=f"bcs{tag}", bufs=1)
                nc.vector.tensor_copy(bcs[:], bc[:])
                for m in range(nchunk):
                    nc.vector.tensor_mul(lat[:, mbase + m, :], lat[:, mbase + m, :], bcs[:])

            # rope on local k_pe (chunk 16, rows 0:64)
            cl = work.tile([ROPE, 2, SSH], F32R, name="cl", bufs=1)
            nc.sync.dma_start(cl[:, 0, :], cosl[:])
            nc.sync.dma_start(cl[:, 1, :], ssinl[:])
            swp = ps([ROPE, SSH], "qacc", "swp", bufs=3)
            nc.tensor.matmul(swp[:], psw[:], lat[:ROPE, 16, :], start=True, stop=True)
            sws = work.tile([ROPE, SSH], F32R, name="sws", bufs=1)
            nc.vector.tensor_mul(sws[:], swp[:], cl[:, 1, :])
            t1 = work.tile([ROPE, SSH], F32R, name="t1", bufs=1)
            nc.vector.tensor_mul(t1[:], lat[:ROPE, 16, :], cl[:, 0, :])
            nc.vector.tensor_add(lat[:ROPE, 16, :], t1[:], sws[:])

            # ================= phase 2: AllGather =================
            ag_in = dram.tile([P, LAT_C * SSH], F32R)
            nc.sync.dma_start(ag_in[:], lat[:].rearrange("p m s -> p (m s)"))

        ag_out = dram.tile([N_CORES, P, LAT_C * SSH], F32R, addr_space="Shared")
        nc.gpsimd.collective_compute(
            "AllGather", mybir.AluOpType.bypass, replica_groups=rg,
            ins=[ag_in.opt()], outs=[ag_out.opt()],
        )
        agv = ag_out[:].rearrange("c p (m s) -> c p m s", m=LAT_C)

        # ================= phase 3: B-projections =================
        with tc.tile_pool(name="att", bufs=1) as att:
            wqs = att.tile([P, N_QAC, HPC * QK], F32R)
            nc.sync.dma_start(wqs[:], wq[:])
            wkvs = att.tile([P, N_KVC, HPC * (NOPE + VDIM)], F32R)
            nc.sync.dma_start(wkvs[:], wkv[:])
            qn = [att.tile([P, SEQ], FLASH_DT, name=f"qn{h}") for h in range(HPC)]
            qp = [att.tile([ROPE, SEQ], F32R, name=f"qp{h}") for h in range(HPC)]
            kn = [att.tile([P, SEQ], FLASH_DT, name=f"kn{h}") for h in range(HPC)]
            kpe3 = att.tile([ROPE, N_CORES, SSH], F32R)
            vv = att.tile([P, N_SC, HPC * VDIM], FLASH_DT)
            ao = [att.tile([P, SEQ], F32R, name=f"ao{h}") for h in range(HPC)]

            nc.sync.dma_start(
                kpe3[:], agv[:, :ROPE, 16, :].rearrange("c p s -> p c s"))
            if FLASH_DT is F32R:
                qpb = qp
                kpe = kpe3[:].rearrange("p c s -> p (c s)")
                onesc_f = ones_col_t
            else:
                qpb = [att.tile([ROPE, SEQ], FLASH_DT, name=f"qpb{h}") for h in range(HPC)]
                kpeb = att.tile([ROPE, SEQ], FLASH_DT)
                with nc.allow_low_precision(reason="flash operands are bf16"):
                    nc.vector.tensor_copy(kpeb[:], kpe3[:].rearrange("p c s -> p (c s)"))
                kpe = kpeb[:]
                onesc_f = att.tile([P, 1], FLASH_DT)
                with nc.allow_low_precision(reason="ones vector"):
                    nc.vector.tensor_copy(onesc_f[:], ones_col)

            # q-projection: k-outer, 4 concurrent accumulators
            for nb in range(N_SB):
                sblk = slice(nb * 512, (nb + 1) * 512)
                accs = []
                for h in range(HPC):
                    accs.append((ps([P, 512], "qacc", "q_acc", bufs=3),
                                 ps([P, 512], "oacc" if h == 0 else "oacc2", "qp_acc")))
                for k in range(N_QAC):
                    qa3 = lstr.tile([P, 2, SSH], F32R, name="qa")
                    c0 = nb * 2
                    nc.sync.dma_start(
                        qa3[:], agv[c0:c0 + 2, :, k, :].rearrange("c p s -> p c s"))
                    qa = qa3[:].rearrange("p c s -> p (c s)")
                    for h in range(HPC):
                        an, ap_ = accs[h]
                        col = h * QK
                        nc.tensor.matmul(an[:], wqs[:, k, col:col + NOPE], qa,
                                         start=(k == 0), stop=(k == N_QAC - 1))
                        nc.tensor.matmul(ap_[:ROPE, :], wqs[:, k, col + NOPE:col + QK],
                                         qa, start=(k == 0), stop=(k == N_QAC - 1))
                for h in range(HPC):
                    an, ap_ = accs[h]
                    with nc.allow_low_precision(reason="flash operands are bf16"):
                        nc.vector.tensor_copy(qn[h][:, sblk], an[:])
                    nc.vector.tensor_copy(qp[h][:, sblk], ap_[:ROPE, :])

            # kv-projection
            for nb in range(N_SB):
                sblk = slice(nb * 512, (nb + 1) * 512)
                kva = []
                for k in range(N_KVC):
                    kt = lstr.tile([P, 2, SSH], F32R, name="kva", bufs=6)
                    c0 = nb * 2
                    nc.sync.dma_start(
                        kt[:], agv[c0:c0 + 2, :, N_QAC + k, :].rearrange("c p s -> p c s"))
                    kva.append(kt[:].rearrange("p c s -> p (c s)"))
                for h in range(HPC):
                    acc = ps([P, 512], "qacc", "kn_acc", bufs=3)
                    for k in range(N_KVC):
                        nc.tensor.matmul(acc[:], wkvs[:, k, h * NOPE:(h + 1) * NOPE],
                                         kva[k], start=(k == 0), stop=(k == N_KVC - 1))
                    with nc.allow_low_precision(reason="flash operands are bf16"):
                        nc.vector.tensor_copy(kn[h][:, sblk], acc[:])
                for tsub in range(4):
                    t_idx = nb * 4 + tsub
                    acc = ps([P, HPC * VDIM], "qacc", "v_acc", bufs=3)
                    for k in range(N_KVC):
                        nc.tensor.matmul(
                            acc[:], kva[k][:, tsub * P:(tsub + 1) * P],
                            wkvs[:, k, HPC * NOPE:], start=(k == 0), stop=(k == N_KVC - 1))
                    with nc.allow_low_precision(reason="flash operands are bf16"):
                        nc.vector.tensor_copy(vv[:, t_idx, :], acc[:])

            # rope on q_pe (per head, rope tables streamed per block)
            for nb in range(N_SB):
                sblk = slice(nb * 512, (nb + 1) * 512)
                cqt = work.tile([ROPE, 2, 512], F32R, name="cqt")
                nc.sync.dma_start(cqt[:, 0, :], cosq[:, sblk])
                nc.sync.dma_start(cqt[:, 1, :], ssinq[:, sblk])
                for h in range(HPC):
                    swp2 = ps([ROPE, 512], "qacc", "swp2", bufs=3)
                    nc.tensor.matmul(swp2[:], psw[:], qp[h][:, sblk], start=True, stop=True)
                    sw2 = work.tile([ROPE, 512], F32R, name="sw2")
                    nc.vector.tensor_mul(sw2[:], swp2[:], cqt[:, 1, :])
                    t2 = work.tile([ROPE, 512], F32R, name="t2")
                    nc.vector.tensor_mul(t2[:], qp[h][:, sblk], cqt[:, 0, :])
                    with nc.allow_low_precision(reason="flash operands are bf16"):
                        nc.vector.tensor_add(qpb[h][:, sblk], t2[:], sw2[:])

            # ============== phase 4: flash attention (causal) ==============
            # b outer / h inner: consecutive blocks use disjoint PSUM tags so
            # one block's Z-reciprocal/eviction hides under the next's matmuls
            for b in range(N_SB):
                for h in range(HPC):
                    sblk = slice(b * 512, (b + 1) * 512)
                    n_tc = 4 * (b + 1)
                    zac = ps([1, 512], "zacc" if h == 0 else "zacc2", "z_acc")
                    oac = ps([P, 512], "oacc" if h == 0 else "oacc2", "o_acc")

                    # software-pipelined: scores/exp for step t+1 are emitted
                    # before the z/pv consumers of step t, so the PE never
                    # waits on the DVE-mask -> ACT-exp chain.
                    exq = []

                    def emit_scores(t):
                        tsl = slice(t * P, (t + 1) * P)
                        sacc = ps([P, 512], "qacc", "s_acc", bufs=3)
                        nc.tensor.matmul(sacc[:], kn[h][:, tsl], qn[h][:, sblk],
                                         start=True, stop=False)
                        nc.tensor.matmul(sacc[:], kpe[:, tsl], qpb[h][:, sblk],
                                         start=False, stop=True)
                        j = t - 4 * b
                        if j >= 0:
                            nc.vector.tensor_add(sacc[:], sacc[:], masks[:, j, :])
                        ex = epool.tile([P, 512], FLASH_DT, name="ex")
                        nc.scalar.activation(ex[:], sacc[:], Exp, scale=SCALE)
                        exq.append(ex)

                    def emit_consume(t):
                        ex = exq.pop(0)
                        nc.tensor.matmul(zac[:], onesc_f[:], ex[:],
                                         start=(t == 0), stop=(t == n_tc - 1))
                        nc.tensor.matmul(oac[:], vv[:, t, h * VDIM:(h + 1) * VDIM],
                                         ex[:], start=(t == 0), stop=(t == n_tc - 1))

                    emit_scores(0)
                    for t in range(n_tc):
                        if t + 1 < n_tc:
                            emit_scores(t + 1)
                        emit_consume(t)
                    rz = work.tile([1, 512], F32R, name="rz")
                    with nc.allow_low_precision(reason="float32r is bitwise float32"):
                        nc.vector.reciprocal(rz[:], zac[:])
                    bcz = ps([P, 512], "bcast", "bcz")
                    nc.tensor.matmul(bcz[:], ones_row[:], rz[:], start=True, stop=True)
                    bczs = work.tile([P, 512], F32, name="bczs")
                    nc.vector.tensor_copy(bczs[:], bcz[:])
                    nc.vector.tensor_mul(ao[h][:, sblk], oac[:], bczs[:])

            # ================= phase 5: o_proj partial =================
            for nb in range(N_SB):
                osl = slice(nb * 512, (nb + 1) * 512)
                wot = work.tile([P, HPC, 512], F32R, name="wot")
                nc.sync.dma_start(wot[:], wo[:, :, osl])
                for sc in range(N_SC):
                    ssl = slice(sc * P, (sc + 1) * P)
                    acc = ps([P, 512], "qacc", "oo_acc", bufs=3)
                    for kh in range(HPC):
                        nc.tensor.matmul(acc[:], ao[kh][:, ssl], wot[:, kh, :],
                                         start=(kh == 0), stop=(kh == HPC - 1))
                    ot = work.tile([P, 512], F32, name="ot", bufs=3)
                    nc.vector.tensor_copy(ot[:], acc[:])
                    nc.sync.dma_start(yout[ssl, osl], ot[:])


_CACHED = None


def _get_program():
    global _CACHED
    if _CACHED is None:
        _CACHED = build_program()
    return _CACHED


def _host_prep(hidden_states, w_qkv_a, q_a_ln_w, w_q_b, w_kv_b, kv_a_ln_w, w_o,
               positions):
    f32 = np.float32
    hs = np.asarray(hidden_states, dtype=f32)
    w1m = np.asarray(w_qkv_a, dtype=f32)
    wqm = np.asarray(w_q_b, dtype=f32) * np.asarray(q_a_ln_w, f32)[None, :]
    wkvm = np.asarray(w_kv_b, dtype=f32) * np.asarray(kv_a_ln_w, f32)[None, :]
    wom = np.asarray(w_o, dtype=f32)

    # rope tables (interleaved / non-neox), matching the reference fp32 math
    pos = np.asarray(positions).astype(f32)
    inv_freq = (1.0 / (f32(THETA) ** (np.arange(0, ROPE, 2, dtype=f32) / f32(ROPE)))).astype(f32)
    fr = pos[None, :] * inv_freq[:, None]              # [32, S]
    cos = np.cos(fr).astype(f32)
    sin = np.sin(fr).astype(f32)
    cosT = np.repeat(cos, 2, axis=0)                   # [64, S]
    ssinT = np.empty((ROPE, SEQ), f32)
    ssinT[0::2] = -sin
    ssinT[1::2] = sin
    psw = np.zeros((ROPE, ROPE), f32)                  # lhsT: out = psw.T @ x
    for i in range(0, ROPE, 2):
        psw[i + 1, i] = 1.0                            # out[i]   = x[i+1]
        psw[i, i + 1] = 1.0                            # out[i+1] = x[i]

    hT = hs.T                                          # [I, S]
    # pad w_qkv_a^T out-dim 2112 -> 2176 (17*128); cols past 2112 are zero.
    # One 1MB DMA per output chunk m: w1l[m, p, k*128+j] = w1T[k*128+p, m*128+j]
    # so the (m, k) lhsT block is w1l[m][:, k*128:(k+1)*128].
    w1T = np.zeros((HIDDEN, LAT_C * P), f32)
    w1T[:, :QLR + KVLR + ROPE] = w1m.T
    w1l = np.ascontiguousarray(
        w1T.reshape(N_KC, P, LAT_C, P).transpose(2, 1, 0, 3).reshape(LAT_C, P, HIDDEN))
    wq4 = wqm.reshape(H, QK, QLR)
    wkv4 = wkvm.reshape(H, NOPE + VDIM, KVLR)

    in_maps = []
    for c in range(N_CORES):
        ssl = slice(c * SSH, (c + 1) * SSH)
        h1 = np.ascontiguousarray(hT[:, ssl].reshape(N_KC, P, SSH).transpose(1, 0, 2))
        wqc = wq4[HPC * c:HPC * (c + 1)].reshape(HPC * QK, QLR).T   # [QLR, 384]
        wql = np.ascontiguousarray(wqc.reshape(N_QAC, P, HPC * QK).transpose(1, 0, 2))
        # column order per k-chunk: [kn_h0 | kn_h1 | v_h0 | v_h1]
        wkvc = wkv4[HPC * c:HPC * (c + 1)]                          # [2, 256, 512]
        wkv_cols = np.concatenate([wkvc[0, :NOPE], wkvc[1, :NOPE],
                                   wkvc[0, NOPE:], wkvc[1, NOPE:]], axis=0)  # [512, KVLR]
        wkvT = wkv_cols.T                                           # [KVLR, 512]
        wkvl = np.ascontiguousarray(
            wkvT.reshape(N_KVC, P, HPC * (NOPE + VDIM)).transpose(1, 0, 2))
        woc = wom[:, HPC * VDIM * c:HPC * VDIM * (c + 1)].T          # [256, 2048]
        wol = np.ascontiguousarray(woc.reshape(HPC, P, HIDDEN).transpose(1, 0, 2))
        in_maps.append({
            "h1": h1, "w1": w1l, "wq": wql, "wkv": wkvl, "wo": wol,
            "cosq": cosT, "ssinq": ssinT,
            "cosl": np.ascontiguousarray(cosT[:, ssl]),
            "ssinl": np.ascontiguousarray(ssinT[:, ssl]),
            "pswap": psw,
            "onesc": np.ones((P, 1), f32),
            "onesr": np.ones((1, P), f32),
        })
    return in_maps


def kernel(**inputs):
    nc = _get_program()
    in_maps = _host_prep(**inputs)
    res = run_bass_kernel_spmd(nc, in_maps, list(range(N_CORES)))
    out = np.zeros((SEQ, HIDDEN), np.float64)
    for c in range(N_CORES):
        out += res.results[c]["y"].astype(np.float64)
    return out.astype(np.float32)


# revision 62
# speedup vs baseline: 1.1179x; 1.1179x over previous
"""DeepseekV2 MLA decoder-layer attention on 8 Trainium2 NeuronCores.

Distribution (tensor-parallel over heads, per the source hint):
  - A-projection (hidden @ w_qkv_a.T) is sequence-sharded: each core computes
    the fused low-rank latents for its 256-token shard, applies the rmsnorms
    (ln weights folded into the B-projection weights on host) and the k_pe
    RoPE, then one AllGather replicates the full latent matrix.
  - B-projections, RoPE(q_pe), flash-style causal attention and o_proj are
    head-sharded: core c owns heads {2c, 2c+1}; its o_proj against the matching
    w_o column slice yields a partial [2048, 2048] output.
  - Unshard on host: output = sum of the 8 partials (RowParallel reduction).

All matmuls run in float32r (~1.7e-4 scale-relative max error per 2048-deep
contraction, full PE throughput).

Layout conventions on device (partition dim first):
  activations feature-major [d, s] so matmuls contract on partitions;
  v is token-major [t, (h, vdim)] so PV contracts over keys;
  scores are computed transposed [t_chunk, s_block]; softmax row sums via
  ones-vector matmuls; normalization deferred to after PV.
"""
import numpy as np

import concourse.bass as bass
import concourse.mybir as mybir
import concourse.tile as tile
from concourse import bacc
from concourse.bass_utils import run_bass_kernel_spmd

HIDDEN = 2048
H = 16
NOPE = 128
ROPE = 64
VDIM = 128
QLR = 1536
KVLR = 512
QK = NOPE + ROPE            # 192
THETA = 10000.0
EPS = 1e-6
SEQ = 2048

N_CORES = 8
HPC = H // N_CORES          # 2 heads per core
SSH = SEQ // N_CORES        # 256-token shard
LAT_C = 17                  # latent chunks: 12 q_a + 4 kv_a + 1 (kpe, 64 rows)
P = 128

F32 = mybir.dt.float32
F32R = mybir.dt.float32r
BF16 = mybir.dt.bfloat16
F16 = mybir.dt.float16
FLASH_DT = F16              # dtype of q/k/v/exp inside flash attention
LAT_DT = F16                # dtype of the AllGather payload + B-proj operands
EXP_BIAS = -4.0             # exp(x*scale + EXP_BIAS): cancels in softmax ratio,
                            # keeps fp16 exp values in range

SCALE = float(QK) ** -0.5
NEG = -1.0e30

N_KC = HIDDEN // P          # 16
N_QAC = QLR // P            # 12
N_KVC = KVLR // P           # 4
N_SB = SEQ // 512           # 4 query blocks
N_SC = SEQ // P             # 16


def build_program():
    nc = bacc.Bacc("TRN2", target_bir_lowering=False, debug=False,
                   num_devices=N_CORES)

    h1 = nc.dram_tensor("h1", [P, N_KC, SSH], F32R, kind="ExternalInput")
    w1 = nc.dram_tensor("w1", [LAT_C, P, HIDDEN], F32R, kind="ExternalInput")
    wq = nc.dram_tensor("wq", [P, N_QAC, HPC * QK], LAT_DT, kind="ExternalInput")
    wkv = nc.dram_tensor("wkv", [P, N_KVC, HPC * (NOPE + VDIM)], LAT_DT, kind="ExternalInput")
    wo = nc.dram_tensor("wo", [P, HPC, HIDDEN], F32R, kind="ExternalInput")
    cosq = nc.dram_tensor("cosq", [ROPE, SEQ], F32R, kind="ExternalInput")
    ssinq = nc.dram_tensor("ssinq", [ROPE, SEQ], F32R, kind="ExternalInput")
    cosl = nc.dram_tensor("cosl", [ROPE, SSH], F32R, kind="ExternalInput")
    ssinl = nc.dram_tensor("ssinl", [ROPE, SSH], F32R, kind="ExternalInput")
    pswap = nc.dram_tensor("pswap", [ROPE, ROPE], F32R, kind="ExternalInput")
    onesc_d = nc.dram_tensor("onesc", [P, 1], F32R, kind="ExternalInput")
    onesr_d = nc.dram_tensor("onesr", [1, P], F32R, kind="ExternalInput")
    yout = nc.dram_tensor("y", [SEQ, HIDDEN], F32, kind="ExternalOutput")

    with tile.TileContext(nc) as tc:
        _emit(nc, tc, h1, w1, wq, wkv, wo, cosq, ssinq, cosl, ssinl, pswap,
              onesc_d, onesr_d, yout)
    nc.compile()
    return nc


def _emit(nc, tc, h1, w1, wq, wkv, wo, cosq, ssinq, cosl, ssinl, pswap,
          onesc_d, onesr_d, yout):
    Exp = mybir.ActivationFunctionType.Exp
    Sqrt = mybir.ActivationFunctionType.Sqrt
    rg = [list(range(N_CORES))]

    with tc.tile_pool(name="const", bufs=1) as const, \
         tc.tile_pool(name="work", bufs=2) as work, \
         tc.tile_pool(name="lstr", bufs=3) as lstr, \
         tc.tile_pool(name="epool", bufs=3) as epool, \
         tc.tile_pool(name="psum", bufs=1, space="PSUM") as psum, \
         tc.tile_pool(name="dram", bufs=1, space="DRAM") as dram:

        # PSUM tags (8 banks): qacc x3, oacc, oacc2, zacc, zacc2, bcast
        def ps(shape, tag, name, bufs=None):
            return psum.tile(shape, F32, tag=tag, name=name, bufs=bufs)

        # ---- constants ----
        ones_col_t = const.tile([P, 1], F32R)
        nc.sync.dma_start(ones_col_t[:], onesc_d[:])
        ones_col = ones_col_t[:]
        ones_row_t = const.tile([1, P], F32R)
        nc.sync.dma_start(ones_row_t[:], onesr_d[:])
        ones_row = ones_row_t[:]
        psw = const.tile([ROPE, ROPE], F32R)
        nc.sync.dma_start(psw[:], pswap[:])
        eps1 = const.tile([1, 1], F32)
        nc.vector.memset(eps1[:], EPS)
        negc = const.tile([P, 1], F32)
        nc.vector.memset(negc[:], EXP_BIAS)
        masks = const.tile([P, 4, 512], F32)
        for j in range(4):
            nc.vector.memset(masks[:, j, :], 0.0)
            nc.gpsimd.affine_select(
                out=masks[:, j, :], in_=masks[:, j, :],
                compare_op=mybir.AluOpType.is_ge, fill=NEG,
                base=-128 * j, pattern=[[1, 512]], channel_multiplier=-1,
            )

        # ================= phase 1: local A-proj + rmsnorm + kpe rope =======
        with tc.tile_pool(name="ph1", bufs=1) as ph1:
            hloc = ph1.tile([P, N_KC, SSH], F32R)
            for k in range(N_KC):
                nc.sync.dma_start(hloc[:, k, :], h1[:, k, :])
            lat = ph1.tile([P, LAT_C, SSH], F32R)
            ss_q = ps([1, SSH], "zacc", "ss_q")
            ss_kv = ps([1, SSH], "zacc2", "ss_kv")

            for m in range(17):
                acc = ps([P, SSH], "qacc", "a_acc", bufs=3)
                wt = ph1.tile([P, HIDDEN], F32R, name="w1t", bufs=3)
                nc.sync.dma_start(wt[:], w1[m])
                for k in range(N_KC):
                    nc.tensor.matmul(acc[:], wt[:, k * P:(k + 1) * P], hloc[:, k, :],
                                     start=(k == 0), stop=(k == N_KC - 1))
                nc.vector.tensor_copy(lat[:, m, :], acc[:])
                if m < 16:
                    sq = work.tile([P, SSH], F32R, name="sq")
                    nc.scalar.square(sq[:], acc[:])
                    tgt = ss_q if m < N_QAC else ss_kv
                    nc.tensor.matmul(tgt[:], ones_col[:], sq[:],
                                     start=(m == 0) or (m == N_QAC),
                                     stop=(m == N_QAC - 1) or (m == 15))

            for tag, ss, nchunk, mbase, denom in (
                ("q", ss_q, N_QAC, 0, QLR), ("kv", ss_kv, N_KVC, N_QAC, KVLR),
            ):
                rt = work.tile([1, SSH], F32, name=f"rt{tag}", bufs=1)
                nc.scalar.activation(rt[:], ss[:], Sqrt, bias=eps1[:], scale=1.0 / denom)
                ri = work.tile([1, SSH], F32R, name=f"ri{tag}", bufs=1)
                with nc.allow_low_precision(reason="float32r is bitwise float32"):
                    nc.vector.reciprocal(ri[:], rt[:])
                bc = ps([P, SSH], "bcast", f"bc{tag}")
                nc.tensor.matmul(bc[:], ones_row[:], ri[:], start=True, stop=True)
                bcs = work.tile([P, SSH], F32, name=f"bcs{tag}", bufs=1)
                nc.vector.tensor_copy(bcs[:], bc[:])
                for m in range(nchunk):
                    nc.vector.tensor_mul(lat[:, mbase + m, :], lat[:, mbase + m, :], bcs[:])

            # rope on local k_pe (chunk 16, rows 0:64)
            cl = work.tile([ROPE, 2, SSH], F32R, name="cl", bufs=1)
            nc.sync.dma_start(cl[:, 0, :], cosl[:])
            nc.sync.dma_start(cl[:, 1, :], ssinl[:])
            swp = ps([ROPE, SSH], "qacc", "swp", bufs=3)
            nc.tensor.matmul(swp[:], psw[:], lat[:ROPE, 16, :], start=True, stop=True)
            sws = work.tile([ROPE, SSH], F32R, name="sws", bufs=1)
            nc.vector.tensor_mul(sws[:], swp[:], cl[:, 1, :])
            t1 = work.tile([ROPE, SSH], F32R, name="t1", bufs=1)
            nc.vector.tensor_mul(t1[:], lat[:ROPE, 16, :], cl[:, 0, :])
            nc.vector.tensor_add(lat[:ROPE, 16, :], t1[:], sws[:])

            # ================= phase 2: AllGather (fp16 payload) ===========
            latf = ph1.tile([P, LAT_C, SSH], LAT_DT)
            with nc.allow_low_precision(reason="fp16 AllGather payload"):
                nc.vector.tensor_copy(latf[:], lat[:])
            ag_in = dram.tile([P, LAT_C * SSH], LAT_DT)
            nc.sync.dma_start(ag_in[:], latf[:].rearrange("p m s -> p (m s)"))

        ag_out = dram.tile([N_CORES, P, LAT_C * SSH], LAT_DT, addr_space="Shared")
        nc.gpsimd.collective_compute(
            "AllGather", mybir.AluOpType.bypass, replica_groups=rg,
            ins=[ag_in.opt()], outs=[ag_out.opt()],
        )
        agv = ag_out[:].rearrange("c p (m s) -> c p m s", m=LAT_C)

        # ================= phase 3: B-projections =================
        with tc.tile_pool(name="att", bufs=1) as att:
            wqs = att.tile([P, N_QAC, HPC * QK], LAT_DT)
            nc.sync.dma_start(wqs[:], wq[:])
            wkvs = att.tile([P, N_KVC, HPC * (NOPE + VDIM)], LAT_DT)
            nc.sync.dma_start(wkvs[:], wkv[:])
            qn = [att.tile([P, SEQ], FLASH_DT, name=f"qn{h}") for h in range(HPC)]
            qp = [att.tile([ROPE, SEQ], F32R, name=f"qp{h}") for h in range(HPC)]
            kn = [att.tile([P, SEQ], FLASH_DT, name=f"kn{h}") for h in range(HPC)]
            kpe3 = att.tile([ROPE, N_CORES, SSH], LAT_DT)
            vv = att.tile([P, N_SC, HPC * VDIM], FLASH_DT)
            ao = [att.tile([P, SEQ], F32R, name=f"ao{h}") for h in range(HPC)]

            nc.sync.dma_start(
                kpe3[:], agv[:, :ROPE, 16, :].rearrange("c p s -> p c s"))
            assert FLASH_DT is LAT_DT
            qpb = [att.tile([ROPE, SEQ], FLASH_DT, name=f"qpb{h}") for h in range(HPC)]
            kpe = kpe3[:].rearrange("p c s -> p (c s)")
            onesc_f = att.tile([P, 1], FLASH_DT)
            with nc.allow_low_precision(reason="ones vector"):
                nc.vector.tensor_copy(onesc_f[:], ones_col)

            # q-projection: k-outer, 4 concurrent accumulators
            for nb in range(N_SB):
                sblk = slice(nb * 512, (nb + 1) * 512)
                accs = []
                for h in range(HPC):
                    accs.append((ps([P, 512], "qacc", "q_acc", bufs=3),
                                 ps([P, 512], "oacc" if h == 0 else "oacc2", "qp_acc")))
                for k in range(N_QAC):
                    qa3 = lstr.tile([P, 2, SSH], LAT_DT, name="qa")
                    c0 = nb * 2
                    nc.sync.dma_start(
                        qa3[:], agv[c0:c0 + 2, :, k, :].rearrange("c p s -> p c s"))
                    qa = qa3[:].rearrange("p c s -> p (c s)")
                    for h in range(HPC):
                        an, ap_ = accs[h]
                        col = h * QK
                        nc.tensor.matmul(an[:], wqs[:, k, col:col + NOPE], qa,
                                         start=(k == 0), stop=(k == N_QAC - 1))
                        nc.tensor.matmul(ap_[:ROPE, :], wqs[:, k, col + NOPE:col + QK],
                                         qa, start=(k == 0), stop=(k == N_QAC - 1))
                for h in range(HPC):
                    an, ap_ = accs[h]
                    with nc.allow_low_precision(reason="flash operands are bf16"):
                        nc.vector.tensor_copy(qn[h][:, sblk], an[:])
                    nc.vector.tensor_copy(qp[h][:, sblk], ap_[:ROPE, :])

            # kv-projection
            for nb in range(N_SB):
                sblk = slice(nb * 512, (nb + 1) * 512)
                kva = []
                for k in range(N_KVC):
                    kt = lstr.tile([P, 2, SSH], LAT_DT, name="kva", bufs=6)
                    c0 = nb * 2
                    nc.sync.dma_start(
                        kt[:], agv[c0:c0 + 2, :, N_QAC + k, :].rearrange("c p s -> p c s"))
                    kva.append(kt[:].rearrange("p c s -> p (c s)"))
                for h in range(HPC):
                    acc = ps([P, 512], "qacc", "kn_acc", bufs=3)
                    for k in range(N_KVC):
                        nc.tensor.matmul(acc[:], wkvs[:, k, h * NOPE:(h + 1) * NOPE],
                                         kva[k], start=(k == 0), stop=(k == N_KVC - 1))
                    with nc.allow_low_precision(reason="flash operands are bf16"):
                        nc.vector.tensor_copy(kn[h][:, sblk], acc[:])
                for tsub in range(4):
                    t_idx = nb * 4 + tsub
                    acc = ps([P, HPC * VDIM], "qacc", "v_acc", bufs=3)
                    for k in range(N_KVC):
                        nc.tensor.matmul(
                            acc[:], kva[k][:, tsub * P:(tsub + 1) * P],
                            wkvs[:, k, HPC * NOPE:], start=(k == 0), stop=(k == N_KVC - 1))
                    with nc.allow_low_precision(reason="flash operands are bf16"):
                        nc.vector.tensor_copy(vv[:, t_idx, :], acc[:])

            # rope on q_pe (per head, rope tables streamed per block)
            for nb in range(N_SB):
                sblk = slice(nb * 512, (nb + 1) * 512)
                cqt = work.tile([ROPE, 2, 512], F32R, name="cqt")
                nc.sync.dma_start(cqt[:, 0, :], cosq[:, sblk])
                nc.sync.dma_start(cqt[:, 1, :], ssinq[:, sblk])
                for h in range(HPC):
                    swp2 = ps([ROPE, 512], "qacc", "swp2", bufs=3)
                    nc.tensor.matmul(swp2[:], psw[:], qp[h][:, sblk], start=True, stop=True)
                    sw2 = work.tile([ROPE, 512], F32R, name="sw2")
                    nc.vector.tensor_mul(sw2[:], swp2[:], cqt[:, 1, :])
                    t2 = work.tile([ROPE, 512], F32R, name="t2")
                    nc.vector.tensor_mul(t2[:], qp[h][:, sblk], cqt[:, 0, :])
                    with nc.allow_low_precision(reason="flash operands are bf16"):
                        nc.vector.tensor_add(qpb[h][:, sblk], t2[:], sw2[:])

            # ============== phase 4: flash attention (causal) ==============
            # b outer / h inner: consecutive blocks use disjoint PSUM tags so
            # one block's Z-reciprocal/eviction hides under the next's matmuls
            for b in range(N_SB):
                for h in range(HPC):
                    sblk = slice(b * 512, (b + 1) * 512)
                    n_tc = 4 * (b + 1)
                    zac = ps([1, 512], "zacc" if h == 0 else "zacc2", "z_acc")
                    oac = ps([P, 512], "oacc" if h == 0 else "oacc2", "o_acc")

                    # software-pipelined: scores/exp for step t+1 are emitted
                    # before the z/pv consumers of step t, so the PE never
                    # waits on the DVE-mask -> ACT-exp chain.
                    exq = []

                    def emit_scores(t):
                        tsl = slice(t * P, (t + 1) * P)
                        sacc = ps([P, 512], "qacc", "s_acc", bufs=3)
                        nc.tensor.matmul(sacc[:], kn[h][:, tsl], qn[h][:, sblk],
                                         start=True, stop=False)
                        nc.tensor.matmul(sacc[:], kpe[:, tsl], qpb[h][:, sblk],
                                         start=False, stop=True)
                        j = t - 4 * b
                        if j >= 0:
                            nc.vector.tensor_add(sacc[:], sacc[:], masks[:, j, :])
                        ex = epool.tile([P, 512], FLASH_DT, name="ex")
                        nc.scalar.activation(ex[:], sacc[:], Exp, scale=SCALE,
                                             bias=negc[:])
                        exq.append(ex)

                    def emit_consume(t):
                        ex = exq.pop(0)
                        nc.tensor.matmul(zac[:], onesc_f[:], ex[:],
                                         start=(t == 0), stop=(t == n_tc - 1))
                        nc.tensor.matmul(oac[:], vv[:, t, h * VDIM:(h + 1) * VDIM],
                                         ex[:], start=(t == 0), stop=(t == n_tc - 1))

                    emit_scores(0)
                    for t in range(n_tc):
                        if t + 1 < n_tc:
                            emit_scores(t + 1)
                        emit_consume(t)
                    rz = work.tile([1, 512], F32R, name="rz")
                    with nc.allow_low_precision(reason="float32r is bitwise float32"):
                        nc.vector.reciprocal(rz[:], zac[:])
                    bcz = ps([P, 512], "bcast", "bcz")
                    nc.tensor.matmul(bcz[:], ones_row[:], rz[:], start=True, stop=True)
                    bczs = work.tile([P, 512], F32, name="bczs")
                    nc.vector.tensor_copy(bczs[:], bcz[:])
                    nc.vector.tensor_mul(ao[h][:, sblk], oac[:], bczs[:])

            # ================= phase 5: o_proj partial =================
            for nb in range(N_SB):
                osl = slice(nb * 512, (nb + 1) * 512)
                wot = work.tile([P, HPC, 512], F32R, name="wot")
                nc.sync.dma_start(wot[:], wo[:, :, osl])
                for sc in range(N_SC):
                    ssl = slice(sc * P, (sc + 1) * P)
                    acc = ps([P, 512], "qacc", "oo_acc", bufs=3)
                    for kh in range(HPC):
                        nc.tensor.matmul(acc[:], ao[kh][:, ssl], wot[:, kh, :],
                                         start=(kh == 0), stop=(kh == HPC - 1))
                    ot = work.tile([P, 512], F32, name="ot", bufs=3)
                    nc.vector.tensor_copy(ot[:], acc[:])
                    nc.sync.dma_start(yout[ssl, osl], ot[:])


_CACHED = None


def _get_program():
    global _CACHED
    if _CACHED is None:
        _CACHED = build_program()
    return _CACHED


def _host_prep(hidden_states, w_qkv_a, q_a_ln_w, w_q_b, w_kv_b, kv_a_ln_w, w_o,
               positions):
    f32 = np.float32
    hs = np.asarray(hidden_states, dtype=f32)
    w1m = np.asarray(w_qkv_a, dtype=f32)
    wqm = np.asarray(w_q_b, dtype=f32) * np.asarray(q_a_ln_w, f32)[None, :]
    wkvm = np.asarray(w_kv_b, dtype=f32) * np.asarray(kv_a_ln_w, f32)[None, :]
    wom = np.asarray(w_o, dtype=f32)

    # rope tables (interleaved / non-neox), matching the reference fp32 math
    pos = np.asarray(positions).astype(f32)
    inv_freq = (1.0 / (f32(THETA) ** (np.arange(0, ROPE, 2, dtype=f32) / f32(ROPE)))).astype(f32)
    fr = pos[None, :] * inv_freq[:, None]              # [32, S]
    cos = np.cos(fr).astype(f32)
    sin = np.sin(fr).astype(f32)
    cosT = np.repeat(cos, 2, axis=0)                   # [64, S]
    ssinT = np.empty((ROPE, SEQ), f32)
    ssinT[0::2] = -sin
    ssinT[1::2] = sin
    psw = np.zeros((ROPE, ROPE), f32)                  # lhsT: out = psw.T @ x
    for i in range(0, ROPE, 2):
        psw[i + 1, i] = 1.0                            # out[i]   = x[i+1]
        psw[i, i + 1] = 1.0                            # out[i+1] = x[i]

    hT = hs.T                                          # [I, S]
    # pad w_qkv_a^T out-dim 2112 -> 2176 (17*128); cols past 2112 are zero.
    # One 1MB DMA per output chunk m: w1l[m, p, k*128+j] = w1T[k*128+p, m*128+j]
    # so the (m, k) lhsT block is w1l[m][:, k*128:(k+1)*128].
    w1T = np.zeros((HIDDEN, LAT_C * P), f32)
    w1T[:, :QLR + KVLR + ROPE] = w1m.T
    w1l = np.ascontiguousarray(
        w1T.reshape(N_KC, P, LAT_C, P).transpose(2, 1, 0, 3).reshape(LAT_C, P, HIDDEN))
    wq4 = wqm.reshape(H, QK, QLR)
    wkv4 = wkvm.reshape(H, NOPE + VDIM, KVLR)

    in_maps = []
    for c in range(N_CORES):
        ssl = slice(c * SSH, (c + 1) * SSH)
        h1 = np.ascontiguousarray(hT[:, ssl].reshape(N_KC, P, SSH).transpose(1, 0, 2))
        wqc = wq4[HPC * c:HPC * (c + 1)].reshape(HPC * QK, QLR).T   # [QLR, 384]
        wql = np.ascontiguousarray(
            wqc.reshape(N_QAC, P, HPC * QK).transpose(1, 0, 2)).astype(np.float16)
        # column order per k-chunk: [kn_h0 | kn_h1 | v_h0 | v_h1]
        wkvc = wkv4[HPC * c:HPC * (c + 1)]                          # [2, 256, 512]
        wkv_cols = np.concatenate([wkvc[0, :NOPE], wkvc[1, :NOPE],
                                   wkvc[0, NOPE:], wkvc[1, NOPE:]], axis=0)  # [512, KVLR]
        wkvT = wkv_cols.T                                           # [KVLR, 512]
        wkvl = np.ascontiguousarray(
            wkvT.reshape(N_KVC, P, HPC * (NOPE + VDIM)).transpose(1, 0, 2)).astype(np.float16)
        woc = wom[:, HPC * VDIM * c:HPC * VDIM * (c + 1)].T          # [256, 2048]
        wol = np.ascontiguousarray(woc.reshape(HPC, P, HIDDEN).transpose(1, 0, 2))
        in_maps.append({
            "h1": h1, "w1": w1l, "wq": wql, "wkv": wkvl, "wo": wol,
            "cosq": cosT, "ssinq": ssinT,
            "cosl": np.ascontiguousarray(cosT[:, ssl]),
            "ssinl": np.ascontiguousarray(ssinT[:, ssl]),
            "pswap": psw,
            "onesc": np.ones((P, 1), f32),
            "onesr": np.ones((1, P), f32),
        })
    return in_maps


def kernel(**inputs):
    nc = _get_program()
    in_maps = _host_prep(**inputs)
    res = run_bass_kernel_spmd(nc, in_maps, list(range(N_CORES)))
    out = np.zeros((SEQ, HIDDEN), np.float64)
    for c in range(N_CORES):
        out += res.results[c]["y"].astype(np.float64)
    return out.astype(np.float32)


# revision 66
# speedup vs baseline: 1.1393x; 1.0191x over previous
"""DeepseekV2 MLA decoder-layer attention on 8 Trainium2 NeuronCores.

Distribution (tensor-parallel over heads, per the source hint):
  - A-projection (hidden @ w_qkv_a.T) is sequence-sharded: each core computes
    the fused low-rank latents for its 256-token shard, applies the rmsnorms
    (ln weights folded into the B-projection weights on host) and the k_pe
    RoPE, then one AllGather replicates the full latent matrix.
  - B-projections, RoPE(q_pe), flash-style causal attention and o_proj are
    head-sharded: core c owns heads {2c, 2c+1}; its o_proj against the matching
    w_o column slice yields a partial [2048, 2048] output.
  - Unshard on host: output = sum of the 8 partials (RowParallel reduction).

All matmuls run in float32r (~1.7e-4 scale-relative max error per 2048-deep
contraction, full PE throughput).

Layout conventions on device (partition dim first):
  activations feature-major [d, s] so matmuls contract on partitions;
  v is token-major [t, (h, vdim)] so PV contracts over keys;
  scores are computed transposed [t_chunk, s_block]; softmax row sums via
  ones-vector matmuls; normalization deferred to after PV.
"""
import numpy as np

import concourse.bass as bass
import concourse.mybir as mybir
import concourse.tile as tile
from concourse import bacc
from concourse.bass_utils import run_bass_kernel_spmd

HIDDEN = 2048
H = 16
NOPE = 128
ROPE = 64
VDIM = 128
QLR = 1536
KVLR = 512
QK = NOPE + ROPE            # 192
THETA = 10000.0
EPS = 1e-6
SEQ = 2048

N_CORES = 8
HPC = H // N_CORES          # 2 heads per core
SSH = SEQ // N_CORES        # 256-token shard
LAT_C = 17                  # latent chunks: 12 q_a + 4 kv_a + 1 (kpe, 64 rows)
P = 128

F32 = mybir.dt.float32
F32R = mybir.dt.float32r
BF16 = mybir.dt.bfloat16
F16 = mybir.dt.float16
FLASH_DT = F16              # dtype of q/k/v/exp inside flash attention
LAT_DT = F16                # dtype of the AllGather payload + B-proj operands
EXP_BIAS = -4.0             # exp(x*scale + EXP_BIAS): cancels in softmax ratio,
                            # keeps fp16 exp values in range

SCALE = float(QK) ** -0.5
NEG = -1.0e30

N_KC = HIDDEN // P          # 16
N_QAC = QLR // P            # 12
N_KVC = KVLR // P           # 4
N_SB = SEQ // 512           # 4 query blocks
N_SC = SEQ // P             # 16


def build_program():
    nc = bacc.Bacc("TRN2", target_bir_lowering=False, debug=False,
                   num_devices=N_CORES)

    h1 = nc.dram_tensor("h1", [P, N_KC, SSH], F32R, kind="ExternalInput")
    w1 = nc.dram_tensor("w1", [LAT_C, P, HIDDEN], F32R, kind="ExternalInput")
    wq = nc.dram_tensor("wq", [P, N_QAC, HPC * QK], LAT_DT, kind="ExternalInput")
    wkv = nc.dram_tensor("wkv", [P, N_KVC, HPC * (NOPE + VDIM)], LAT_DT, kind="ExternalInput")
    wo = nc.dram_tensor("wo", [P, HPC, HIDDEN], F32R, kind="ExternalInput")
    cosq = nc.dram_tensor("cosq", [ROPE, SEQ], F32R, kind="ExternalInput")
    ssinq = nc.dram_tensor("ssinq", [ROPE, SEQ], F32R, kind="ExternalInput")
    cosl = nc.dram_tensor("cosl", [ROPE, SSH], F32R, kind="ExternalInput")
    ssinl = nc.dram_tensor("ssinl", [ROPE, SSH], F32R, kind="ExternalInput")
    pswap = nc.dram_tensor("pswap", [ROPE, ROPE], F32R, kind="ExternalInput")
    onesc_d = nc.dram_tensor("onesc", [P, 1], F32R, kind="ExternalInput")
    onesr_d = nc.dram_tensor("onesr", [1, P], F32R, kind="ExternalInput")
    yout = nc.dram_tensor("y", [SEQ, HIDDEN], F32, kind="ExternalOutput")

    with tile.TileContext(nc) as tc:
        _emit(nc, tc, h1, w1, wq, wkv, wo, cosq, ssinq, cosl, ssinl, pswap,
              onesc_d, onesr_d, yout)
    nc.compile()
    return nc


def _emit(nc, tc, h1, w1, wq, wkv, wo, cosq, ssinq, cosl, ssinl, pswap,
          onesc_d, onesr_d, yout):
    Exp = mybir.ActivationFunctionType.Exp
    Sqrt = mybir.ActivationFunctionType.Sqrt
    rg = [list(range(N_CORES))]

    with tc.tile_pool(name="const", bufs=1) as const, \
         tc.tile_pool(name="work", bufs=2) as work, \
         tc.tile_pool(name="lstr", bufs=4) as lstr, \
         tc.tile_pool(name="epool", bufs=4) as epool, \
         tc.tile_pool(name="psum", bufs=1, space="PSUM") as psum, \
         tc.tile_pool(name="dram", bufs=1, space="DRAM") as dram:

        # PSUM tags (8 banks): qacc x3, oacc, oacc2, zacc, zacc2, bcast
        def ps(shape, tag, name, bufs=None):
            return psum.tile(shape, F32, tag=tag, name=name, bufs=bufs)

        # ---- constants ----
        ones_col_t = const.tile([P, 1], F32R)
        nc.sync.dma_start(ones_col_t[:], onesc_d[:])
        ones_col = ones_col_t[:]
        ones_row_t = const.tile([1, P], F32R)
        nc.sync.dma_start(ones_row_t[:], onesr_d[:])
        ones_row = ones_row_t[:]
        psw = const.tile([ROPE, ROPE], F32R)
        nc.sync.dma_start(psw[:], pswap[:])
        eps1 = const.tile([1, 1], F32)
        nc.vector.memset(eps1[:], EPS)
        negc = const.tile([P, 1], F32)
        nc.vector.memset(negc[:], EXP_BIAS)
        masks = const.tile([P, 4, 512], F32)
        for j in range(4):
            nc.vector.memset(masks[:, j, :], 0.0)
            nc.gpsimd.affine_select(
                out=masks[:, j, :], in_=masks[:, j, :],
                compare_op=mybir.AluOpType.is_ge, fill=NEG,
                base=-128 * j, pattern=[[1, 512]], channel_multiplier=-1,
            )

        # ================= phase 1: local A-proj + rmsnorm + kpe rope =======
        with tc.tile_pool(name="ph1", bufs=1) as ph1:
            hloc = ph1.tile([P, N_KC, SSH], F32R)
            for k in range(N_KC):
                nc.sync.dma_start(hloc[:, k, :], h1[:, k, :])
            lat = ph1.tile([P, LAT_C, SSH], F32R)
            latf = ph1.tile([P, LAT_C, SSH], LAT_DT)
            ss_q = ps([1, SSH], "zacc", "ss_q")
            ss_kv = ps([1, SSH], "zacc2", "ss_kv")

            for m in range(17):
                acc = ps([P, SSH], "qacc", "a_acc", bufs=3)
                wt = ph1.tile([P, HIDDEN], F32R, name="w1t", bufs=3)
                nc.sync.dma_start(wt[:], w1[m])
                for k in range(N_KC):
                    nc.tensor.matmul(acc[:], wt[:, k * P:(k + 1) * P], hloc[:, k, :],
                                     start=(k == 0), stop=(k == N_KC - 1))
                nc.vector.tensor_copy(lat[:, m, :], acc[:])
                if m < 16:
                    sq = work.tile([P, SSH], F32R, name="sq")
                    nc.scalar.square(sq[:], acc[:])
                    tgt = ss_q if m < N_QAC else ss_kv
                    nc.tensor.matmul(tgt[:], ones_col[:], sq[:],
                                     start=(m == 0) or (m == N_QAC),
                                     stop=(m == N_QAC - 1) or (m == 15))

            for tag, ss, nchunk, mbase, denom in (
                ("q", ss_q, N_QAC, 0, QLR), ("kv", ss_kv, N_KVC, N_QAC, KVLR),
            ):
                rt = work.tile([1, SSH], F32, name=f"rt{tag}", bufs=1)
                nc.scalar.activation(rt[:], ss[:], Sqrt, bias=eps1[:], scale=1.0 / denom)
                ri = work.tile([1, SSH], F32R, name=f"ri{tag}", bufs=1)
                with nc.allow_low_precision(reason="float32r is bitwise float32"):
                    nc.vector.reciprocal(ri[:], rt[:])
                bc = ps([P, SSH], "bcast", f"bc{tag}")
                nc.tensor.matmul(bc[:], ones_row[:], ri[:], start=True, stop=True)
                bcs = work.tile([P, SSH], F32, name=f"bcs{tag}", bufs=1)
                nc.vector.tensor_copy(bcs[:], bc[:])
                for m in range(nchunk):
                    with nc.allow_low_precision(reason="fp16 AllGather payload"):
                        nc.vector.tensor_mul(latf[:, mbase + m, :],
                                             lat[:, mbase + m, :], bcs[:])

            # rope on local k_pe (chunk 16, rows 0:64)
            cl = work.tile([ROPE, 2, SSH], F32R, name="cl", bufs=1)
            nc.sync.dma_start(cl[:, 0, :], cosl[:])
            nc.sync.dma_start(cl[:, 1, :], ssinl[:])
            swp = ps([ROPE, SSH], "qacc", "swp", bufs=3)
            nc.tensor.matmul(swp[:], psw[:], lat[:ROPE, 16, :], start=True, stop=True)
            sws = work.tile([ROPE, SSH], F32R, name="sws", bufs=1)
            nc.vector.tensor_mul(sws[:], swp[:], cl[:, 1, :])
            t1 = work.tile([ROPE, SSH], F32R, name="t1", bufs=1)
            nc.vector.tensor_mul(t1[:], lat[:ROPE, 16, :], cl[:, 0, :])
            with nc.allow_low_precision(reason="fp16 AllGather payload"):
                nc.vector.tensor_add(latf[:ROPE, 16, :], t1[:], sws[:])
                nc.vector.memset(latf[ROPE:, 16, :], 0.0)

            # ================= phase 2: AllGather (fp16 payload) ===========
            ag_in = dram.tile([P, LAT_C * SSH], LAT_DT)
            nc.sync.dma_start(ag_in[:], latf[:].rearrange("p m s -> p (m s)"))

        ag_out = dram.tile([N_CORES, P, LAT_C * SSH], LAT_DT, addr_space="Shared")
        nc.gpsimd.collective_compute(
            "AllGather", mybir.AluOpType.bypass, replica_groups=rg,
            ins=[ag_in.opt()], outs=[ag_out.opt()],
        )
        agv = ag_out[:].rearrange("c p (m s) -> c p m s", m=LAT_C)

        # ================= phase 3: B-projections =================
        with tc.tile_pool(name="att", bufs=1) as att:
            wqs = att.tile([P, N_QAC, HPC * QK], LAT_DT)
            nc.sync.dma_start(wqs[:], wq[:])
            wkvs = att.tile([P, N_KVC, HPC * (NOPE + VDIM)], LAT_DT)
            nc.sync.dma_start(wkvs[:], wkv[:])
            qn = [att.tile([P, SEQ], FLASH_DT, name=f"qn{h}") for h in range(HPC)]
            qp = [att.tile([ROPE, SEQ], F32R, name=f"qp{h}") for h in range(HPC)]
            kn = [att.tile([P, SEQ], FLASH_DT, name=f"kn{h}") for h in range(HPC)]
            kpe3 = att.tile([ROPE, N_CORES, SSH], LAT_DT)
            vv = att.tile([P, N_SC, HPC * VDIM], FLASH_DT)
            ao = [att.tile([P, SEQ], F32R, name=f"ao{h}") for h in range(HPC)]

            nc.sync.dma_start(
                kpe3[:], agv[:, :ROPE, 16, :].rearrange("c p s -> p c s"))
            assert FLASH_DT is LAT_DT
            qpb = [att.tile([ROPE, SEQ], FLASH_DT, name=f"qpb{h}") for h in range(HPC)]
            kpe = kpe3[:].rearrange("p c s -> p (c s)")
            onesc_f = att.tile([P, 1], FLASH_DT)
            with nc.allow_low_precision(reason="ones vector"):
                nc.vector.tensor_copy(onesc_f[:], ones_col)

            # q-projection: k-outer, 4 concurrent accumulators
            for nb in range(N_SB):
                sblk = slice(nb * 512, (nb + 1) * 512)
                accs = []
                for h in range(HPC):
                    accs.append((ps([P, 512], "qacc", "q_acc", bufs=3),
                                 ps([P, 512], "oacc" if h == 0 else "oacc2", "qp_acc")))
                for k in range(N_QAC):
                    qa3 = lstr.tile([P, 2, SSH], LAT_DT, name="qa")
                    c0 = nb * 2
                    nc.sync.dma_start(
                        qa3[:], agv[c0:c0 + 2, :, k, :].rearrange("c p s -> p c s"))
                    qa = qa3[:].rearrange("p c s -> p (c s)")
                    for h in range(HPC):
                        an, ap_ = accs[h]
                        col = h * QK
                        nc.tensor.matmul(an[:], wqs[:, k, col:col + NOPE], qa,
                                         start=(k == 0), stop=(k == N_QAC - 1))
                        nc.tensor.matmul(ap_[:ROPE, :], wqs[:, k, col + NOPE:col + QK],
                                         qa, start=(k == 0), stop=(k == N_QAC - 1))
                for h in range(HPC):
                    an, ap_ = accs[h]
                    with nc.allow_low_precision(reason="flash operands are bf16"):
                        nc.vector.tensor_copy(qn[h][:, sblk], an[:])
                    nc.vector.tensor_copy(qp[h][:, sblk], ap_[:ROPE, :])

            # kv-projection
            for nb in range(N_SB):
                sblk = slice(nb * 512, (nb + 1) * 512)
                kva = []
                for k in range(N_KVC):
                    kt = lstr.tile([P, 2, SSH], LAT_DT, name="kva", bufs=6)
                    c0 = nb * 2
                    nc.sync.dma_start(
                        kt[:], agv[c0:c0 + 2, :, N_QAC + k, :].rearrange("c p s -> p c s"))
                    kva.append(kt[:].rearrange("p c s -> p (c s)"))
                for h in range(HPC):
                    acc = ps([P, 512], "qacc", "kn_acc", bufs=3)
                    for k in range(N_KVC):
                        nc.tensor.matmul(acc[:], wkvs[:, k, h * NOPE:(h + 1) * NOPE],
                                         kva[k], start=(k == 0), stop=(k == N_KVC - 1))
                    with nc.allow_low_precision(reason="flash operands are bf16"):
                        nc.vector.tensor_copy(kn[h][:, sblk], acc[:])
                for tsub in range(4):
                    t_idx = nb * 4 + tsub
                    acc = ps([P, HPC * VDIM], "qacc", "v_acc", bufs=3)
                    for k in range(N_KVC):
                        nc.tensor.matmul(
                            acc[:], kva[k][:, tsub * P:(tsub + 1) * P],
                            wkvs[:, k, HPC * NOPE:], start=(k == 0), stop=(k == N_KVC - 1))
                    with nc.allow_low_precision(reason="flash operands are bf16"):
                        nc.vector.tensor_copy(vv[:, t_idx, :], acc[:])

            # rope on q_pe (per head, rope tables streamed per block)
            for nb in range(N_SB):
                sblk = slice(nb * 512, (nb + 1) * 512)
                cqt = work.tile([ROPE, 2, 512], F32R, name="cqt")
                nc.sync.dma_start(cqt[:, 0, :], cosq[:, sblk])
                nc.sync.dma_start(cqt[:, 1, :], ssinq[:, sblk])
                for h in range(HPC):
                    swp2 = ps([ROPE, 512], "qacc", "swp2", bufs=3)
                    nc.tensor.matmul(swp2[:], psw[:], qp[h][:, sblk], start=True, stop=True)
                    sw2 = work.tile([ROPE, 512], F32R, name="sw2")
                    nc.vector.tensor_mul(sw2[:], swp2[:], cqt[:, 1, :])
                    t2 = work.tile([ROPE, 512], F32R, name="t2")
                    nc.vector.tensor_mul(t2[:], qp[h][:, sblk], cqt[:, 0, :])
                    with nc.allow_low_precision(reason="flash operands are bf16"):
                        nc.vector.tensor_add(qpb[h][:, sblk], t2[:], sw2[:])

            # ============== phase 4: flash attention (causal) ==============
            # b outer / h inner: consecutive blocks use disjoint PSUM tags so
            # one block's Z-reciprocal/eviction hides under the next's matmuls
            for b in range(N_SB):
                for h in range(HPC):
                    sblk = slice(b * 512, (b + 1) * 512)
                    n_tc = 4 * (b + 1)
                    zac = ps([1, 512], "zacc" if h == 0 else "zacc2", "z_acc")
                    oac = ps([P, 512], "oacc" if h == 0 else "oacc2", "o_acc")

                    # software-pipelined: scores/exp for step t+1 are emitted
                    # before the z/pv consumers of step t, so the PE never
                    # waits on the DVE-mask -> ACT-exp chain.
                    exq = []

                    def emit_scores(t):
                        tsl = slice(t * P, (t + 1) * P)
                        sacc = ps([P, 512], "qacc", "s_acc", bufs=3)
                        nc.tensor.matmul(sacc[:], kn[h][:, tsl], qn[h][:, sblk],
                                         start=True, stop=False)
                        nc.tensor.matmul(sacc[:], kpe[:, tsl], qpb[h][:, sblk],
                                         start=False, stop=True)
                        j = t - 4 * b
                        if j >= 0:
                            nc.vector.tensor_add(sacc[:], sacc[:], masks[:, j, :])
                        ex = epool.tile([P, 512], FLASH_DT, name="ex")
                        nc.scalar.activation(ex[:], sacc[:], Exp, scale=SCALE,
                                             bias=negc[:])
                        exq.append(ex)

                    def emit_consume(t):
                        ex = exq.pop(0)
                        nc.tensor.matmul(zac[:], onesc_f[:], ex[:],
                                         start=(t == 0), stop=(t == n_tc - 1))
                        nc.tensor.matmul(oac[:], vv[:, t, h * VDIM:(h + 1) * VDIM],
                                         ex[:], start=(t == 0), stop=(t == n_tc - 1))

                    emit_scores(0)
                    for t in range(n_tc):
                        if t + 1 < n_tc:
                            emit_scores(t + 1)
                        emit_consume(t)
                    rz = work.tile([1, 512], F32R, name="rz")
                    with nc.allow_low_precision(reason="float32r is bitwise float32"):
                        nc.vector.reciprocal(rz[:], zac[:])
                    bcz = ps([P, 512], "bcast", "bcz")
                    nc.tensor.matmul(bcz[:], ones_row[:], rz[:], start=True, stop=True)
                    bczs = work.tile([P, 512], F32, name="bczs")
                    nc.vector.tensor_copy(bczs[:], bcz[:])
                    nc.vector.tensor_mul(ao[h][:, sblk], oac[:], bczs[:])

            # ================= phase 5: o_proj partial =================
            for nb in range(N_SB):
                osl = slice(nb * 512, (nb + 1) * 512)
                wot = work.tile([P, HPC, 512], F32R, name="wot")
                nc.sync.dma_start(wot[:], wo[:, :, osl])
                for sc in range(N_SC):
                    ssl = slice(sc * P, (sc + 1) * P)
                    acc = ps([P, 512], "qacc", "oo_acc", bufs=3)
                    for kh in range(HPC):
                        nc.tensor.matmul(acc[:], ao[kh][:, ssl], wot[:, kh, :],
                                         start=(kh == 0), stop=(kh == HPC - 1))
                    ot = work.tile([P, 512], F32, name="ot", bufs=3)
                    nc.vector.tensor_copy(ot[:], acc[:])
                    nc.sync.dma_start(yout[ssl, osl], ot[:])


_CACHED = None


def _get_program():
    global _CACHED
    if _CACHED is None:
        _CACHED = build_program()
    return _CACHED


def _host_prep(hidden_states, w_qkv_a, q_a_ln_w, w_q_b, w_kv_b, kv_a_ln_w, w_o,
               positions):
    f32 = np.float32
    hs = np.asarray(hidden_states, dtype=f32)
    w1m = np.asarray(w_qkv_a, dtype=f32)
    wqm = np.asarray(w_q_b, dtype=f32) * np.asarray(q_a_ln_w, f32)[None, :]
    wkvm = np.asarray(w_kv_b, dtype=f32) * np.asarray(kv_a_ln_w, f32)[None, :]
    wom = np.asarray(w_o, dtype=f32)

    # rope tables (interleaved / non-neox), matching the reference fp32 math
    pos = np.asarray(positions).astype(f32)
    inv_freq = (1.0 / (f32(THETA) ** (np.arange(0, ROPE, 2, dtype=f32) / f32(ROPE)))).astype(f32)
    fr = pos[None, :] * inv_freq[:, None]              # [32, S]
    cos = np.cos(fr).astype(f32)
    sin = np.sin(fr).astype(f32)
    cosT = np.repeat(cos, 2, axis=0)                   # [64, S]
    ssinT = np.empty((ROPE, SEQ), f32)
    ssinT[0::2] = -sin
    ssinT[1::2] = sin
    psw = np.zeros((ROPE, ROPE), f32)                  # lhsT: out = psw.T @ x
    for i in range(0, ROPE, 2):
        psw[i + 1, i] = 1.0                            # out[i]   = x[i+1]
        psw[i, i + 1] = 1.0                            # out[i+1] = x[i]

    hT = hs.T                                          # [I, S]
    # pad w_qkv_a^T out-dim 2112 -> 2176 (17*128); cols past 2112 are zero.
    # One 1MB DMA per output chunk m: w1l[m, p, k*128+j] = w1T[k*128+p, m*128+j]
    # so the (m, k) lhsT block is w1l[m][:, k*128:(k+1)*128].
    w1T = np.zeros((HIDDEN, LAT_C * P), f32)
    w1T[:, :QLR + KVLR + ROPE] = w1m.T
    w1l = np.ascontiguousarray(
        w1T.reshape(N_KC, P, LAT_C, P).transpose(2, 1, 0, 3).reshape(LAT_C, P, HIDDEN))
    wq4 = wqm.reshape(H, QK, QLR)
    wkv4 = wkvm.reshape(H, NOPE + VDIM, KVLR)

    in_maps = []
    for c in range(N_CORES):
        ssl = slice(c * SSH, (c + 1) * SSH)
        h1 = np.ascontiguousarray(hT[:, ssl].reshape(N_KC, P, SSH).transpose(1, 0, 2))
        wqc = wq4[HPC * c:HPC * (c + 1)].reshape(HPC * QK, QLR).T   # [QLR, 384]
        wql = np.ascontiguousarray(
            wqc.reshape(N_QAC, P, HPC * QK).transpose(1, 0, 2)).astype(np.float16)
        # column order per k-chunk: [kn_h0 | kn_h1 | v_h0 | v_h1]
        wkvc = wkv4[HPC * c:HPC * (c + 1)]                          # [2, 256, 512]
        wkv_cols = np.concatenate([wkvc[0, :NOPE], wkvc[1, :NOPE],
                                   wkvc[0, NOPE:], wkvc[1, NOPE:]], axis=0)  # [512, KVLR]
        wkvT = wkv_cols.T                                           # [KVLR, 512]
        wkvl = np.ascontiguousarray(
            wkvT.reshape(N_KVC, P, HPC * (NOPE + VDIM)).transpose(1, 0, 2)).astype(np.float16)
        woc = wom[:, HPC * VDIM * c:HPC * VDIM * (c + 1)].T          # [256, 2048]
        wol = np.ascontiguousarray(woc.reshape(HPC, P, HIDDEN).transpose(1, 0, 2))
        in_maps.append({
            "h1": h1, "w1": w1l, "wq": wql, "wkv": wkvl, "wo": wol,
            "cosq": cosT, "ssinq": ssinT,
            "cosl": np.ascontiguousarray(cosT[:, ssl]),
            "ssinl": np.ascontiguousarray(ssinT[:, ssl]),
            "pswap": psw,
            "onesc": np.ones((P, 1), f32),
            "onesr": np.ones((1, P), f32),
        })
    return in_maps


def kernel(**inputs):
    nc = _get_program()
    in_maps = _host_prep(**inputs)
    res = run_bass_kernel_spmd(nc, in_maps, list(range(N_CORES)))
    out = np.zeros((SEQ, HIDDEN), np.float64)
    for c in range(N_CORES):
        out += res.results[c]["y"].astype(np.float64)
    return out.astype(np.float32)
